# revision 28
# baseline (speedup 1.0000x reference)
"""Trainium2 Bass kernel for masked causal multi-head self-attention.

Problem shapes (hardcoded): B=2, T=2048, D=1024, H=16, DH=64.

Sharding: 8 cores, tensor-parallel over (batch, head-group):
core c -> batch b = c // 4, head group g = c % 4 (heads 4g..4g+3,
feature slice 256g..256g+256). Each core computes a partial [D, P]
(transposed) output for its batch; the host sums the 4 partials per
batch, scatters rows back to unmasked positions, and transposes.

Token packing: the reference masks both queries and keys with
data_mask and zeroes masked-query output rows. Packing is
order-preserving, so causal attention over only the unmasked tokens
(gather on the host, zero-pad to a static P = multiple of 512,
scatter back after) is exact: masked keys never enter the sequence
and masked-query rows are zeros by construction. With ~50% masking
this halves the attention quadratic work and cuts the projections by
P/T. The last q tile is further trimmed to LC columns (128-multiple
covering the real tokens), which also trims the key range to
KV = P - 512 + LC: k tiles, V-projection tiles, and the last Q/K
projection tile shrink accordingly. P and LC adapt to the mask at
call time (program cached per (P, LC)); typical masks give P=1536,
LC=128 -> ~2.9x less attention area than the unpacked kernel.

Device algorithm (all matmuls bf16, PSUM accumulation f32):
  - a few junk warm-up matmuls on memset SBUF tiles issue at t=0:
    the PE HAM clock-gate needs ~3.4us of sustained activity to
    release 1.2 -> 2.4 GHz, and the projections would otherwise run
    their first half cold.
  - Q^T and K^T project TOGETHER, chunk-major over the contraction
    (D=1024, 8 chunks), in two passes sized to PSUM (pass A: the two
    full q tiles of each = 4 accumulators = 8 banks; pass B: the
    trimmed last tiles). wq/wk ship in per-chunk-interleaved slices
    on the SP queue while xT chunks alternate the Activation/Pool
    queues, so chunk arrival order matches consumption order and the
    PE streams at the ~330GB/s input rate with no head-of-line
    blocking.
  - V projects chunk-major in per-tile chains: tiles 0..3 upfront
    (between the first score pumps so their matmuls overlap the first
    exps), the rest emitted INTO the attention stream as PE filler
    just before first use - the attention inner loop is ScalarE
    (exp)-bound, so the V chains ride in the PE's exp-wait gaps,
    which also keeps the HAM clock-gate from re-throttling.
  - attention runs as one global (j, m, i) stream, four units ahead:
    scores^T [128 k, 2 heads, <=512 q] -> exp on ScalarE (scale=1/8,
    no max subtraction; scores bounded ~8.2 for this input
    distribution) -> causal triangle multiplied into diagonal k tiles
    on DVE -> AV accumulates o'^T [65, 2, <=512] over k tiles (column
    64 of V' = key-validity indicator, so the softmax denominator
    rides the AV matmul). On diagonal tiles the fully-masked column
    range [0, 128r) is never computed, exp'd, or consumed.
  - normalization per (j, m): reciprocal of the sums row + 1e-30 (DVE
    reads PSUM directly), partition_broadcast on GpSimd (kept
    broadcast-only: hardware GpSimd reloads its op library when
    switching op types, and it cannot access PSUM; DVE rejects
    partition-stride-0 APs), two multiplies producing bf16 o_all.
  - out^T partial = Wp_c^T @ o_all in dt-pairs per [128, 2, 512]
    accumulator; one evacuation + one bf16 DMA per pair. When the
    next q tile is full-width (exp-bound), three of the four dt-pairs
    are deferred into its unit stream as more PE filler (their
    accumulators ride the transient psS rotation). Stores ride the SP
    queue during attention so the Activation queue stays exp-only;
    the last tile splits evacuations and stores across both hardware
    queues (no exps remain).

Engine budget during attention: ScalarE = exp only (the bottleneck,
~94% busy mid-stream), PE = scores/AV + V-chain and out_proj filler,
DVE = tri masks + normalization + evacuations, GpSimd = partition
broadcasts only.

Measured on trn2 (8 cores, axon): ~97.3us HW exec (prior session
baseline 175.1us), L2 rel err 5.5e-3 vs the float64 reference (bf16
rounding dominates).
"""

import numpy as np

B, T, D, H = 2, 2048, 1024, 16
DH = D // H          # 64
HPC = 4              # heads per core
DC = HPC * DH        # 256 feature slice per core
NC = 8               # cores
QT = 512             # q tile width
KT = 128             # k tile width (partition dim)
NCH = D // 128       # 8 contraction chunks
SCALE = float(DH) ** -0.5
N_WARM = 5           # junk warm-up matmuls at t=0

_cached = {}


MM_DTYPE = "bfloat16"  # "float32r" (accurate) or "bfloat16" (fast)


def _build_program(P, LC, mm_dtype=None):
    import concourse.tile as tile
    from concourse import bacc, mybir

    NQT = P // QT        # q tiles
    KV = P - QT + LC     # real key span (128-multiple)
    NKT = KV // KT       # k tiles / V tiles
    assert P % QT == 0 and LC % KT == 0 and 0 < LC <= QT

    def jcols(j):        # columns computed for q tile j
        return LC if j == NQT - 1 else QT

    def jkt(j):          # k tiles consumed by q tile j
        return 4 * j + jcols(j) // KT

    F32 = mybir.dt.float32
    MDT = getattr(mybir.dt, mm_dtype or MM_DTYPE)
    EXP = mybir.ActivationFunctionType.Exp

    nc = bacc.Bacc("TRN2", target_bir_lowering=False, debug=False)

    xT_d = nc.dram_tensor("xT", [D, P], MDT, kind="ExternalInput")
    wq_d = nc.dram_tensor("wq", [128, NCH, DC], MDT, kind="ExternalInput")
    wk_d = nc.dram_tensor("wk", [128, NCH, DC], MDT, kind="ExternalInput")
    wv_d = nc.dram_tensor("wv", [128, NCH, DC], MDT, kind="ExternalInput")
    wp_d = nc.dram_tensor("wp", [128, DC // 128, D], MDT, kind="ExternalInput")
    dm01_d = nc.dram_tensor("dm01", [KT, NKT], F32, kind="ExternalInput")
    dm01e_d = nc.dram_tensor("dm01e", [KT, NKT], F32, kind="ExternalInput")
    tri_d = nc.dram_tensor("tri", [KT, KT], MDT, kind="ExternalInput")
    # partials ship as bf16 (the host sums 4 per batch in f32); halves
    # the store traffic and the tail drain
    out_d = nc.dram_tensor("outT", [D, P], MDT, kind="ExternalOutput")

    with tile.TileContext(nc) as tc:
        with (
            tc.tile_pool(name="w", bufs=1) as wpool,
            tc.tile_pool(name="acts", bufs=1) as acts,
            tc.tile_pool(name="wt", bufs=8) as wtp,
            tc.tile_pool(name="sm", bufs=3) as sm,
            tc.tile_pool(name="ob", bufs=6) as obp,
            tc.tile_pool(name="psS", bufs=2, space="PSUM") as psS,
            tc.tile_pool(name="psO", bufs=2, space="PSUM") as psO,
        ):
            # ---- PE warm-up: junk matmuls on memset tiles, issued
            # before any DMA dependency so the HAM clock-gate releases
            # (1.2 -> 2.4 GHz needs ~3.4us of sustained PE activity)
            # while the input DMAs stream ----
            wj = wpool.tile([128, 128], MDT)
            nc.vector.memset(wj[:], 0.0)
            xj = wpool.tile([128, QT], MDT)
            nc.vector.memset(xj[:], 0.0)
            ps_w = psS.tile([128, 2, QT], F32, tag="s", name="ps_warm")
            for _ in range(N_WARM):
                nc.tensor.matmul(ps_w[:, 0, :], wj[:], xj[:],
                                 start=True, stop=True)

            # ---- loads. Issue order feeds the Q projection first;
            # issues spread across four DMA queues so descriptor issue
            # does not serialize the early arrivals ----
            wq = wpool.tile([128, NCH, DC], MDT)
            wk = wpool.tile([128, NCH, DC], MDT)
            wv = wpool.tile([128, NCH, DC], MDT)
            xTs = [wpool.tile([128, KV], MDT, tag=f"xt{kt}", name=f"xt{kt}")
                   for kt in range(NCH)]
            # Q and K project together chunk-major, so wq/wk chunk kt
            # is needed with xT chunk kt: the SP queue ships the two
            # weights in per-chunk-interleaved slices while the xT
            # chunks alternate the other two queues. All three queues
            # share ~330GB/s; the per-chunk PE work (~1.7us) slightly
            # exceeds the per-chunk DMA (~1.6us), so the PE never
            # stalls and the HAM clock-gate stays released.
            nc.sync.dma_start(out=wq[:, 0:1, :], in_=wq_d[:, 0:1, :])
            nc.sync.dma_start(out=wk[:, 0:1, :], in_=wk_d[:, 0:1, :])
            nc.scalar.dma_start(out=xTs[0][:], in_=xT_d[0:128, 0:KV])
            nc.gpsimd.dma_start(out=xTs[1][:], in_=xT_d[128:256, 0:KV])
            nc.sync.dma_start(out=wq[:, 1:4, :], in_=wq_d[:, 1:4, :])
            nc.sync.dma_start(out=wk[:, 1:4, :], in_=wk_d[:, 1:4, :])
            nc.scalar.dma_start(out=xTs[2][:], in_=xT_d[256:384, 0:KV])
            nc.gpsimd.dma_start(out=xTs[3][:], in_=xT_d[384:512, 0:KV])
            nc.sync.dma_start(out=wq[:, 4:NCH, :], in_=wq_d[:, 4:NCH, :])
            nc.sync.dma_start(out=wk[:, 4:NCH, :], in_=wk_d[:, 4:NCH, :])
            for kt in range(4, NCH):
                eng = (nc.scalar, nc.gpsimd)[kt % 2]
                eng.dma_start(out=xTs[kt][:],
                              in_=xT_d[128 * kt:128 * kt + 128, 0:KV])
            nc.gpsimd.dma_start(out=wv[:], in_=wv_d[:])
            tri = wpool.tile([KT, KT], MDT)
            nc.sync.dma_start(out=tri[:], in_=tri_d[:])
            dm01 = wpool.tile([KT, NKT], F32)
            nc.sync.dma_start(out=dm01[:], in_=dm01_d[:])
            # dm01e: the denominator column's key-validity indicator
            dm01e = wpool.tile([KT, NKT], F32)
            nc.sync.dma_start(out=dm01e[:], in_=dm01e_d[:])
            wp = wpool.tile([128, DC // 128, D], MDT)
            nc.scalar.dma_start(out=wp[:], in_=wp_d[:])
            ones4 = wpool.tile([128, HPC], F32)
            nc.vector.memset(ones4[:], 1.0)
            # preload the Exp activation table while the PE projects
            actwarm = wpool.tile([128, HPC], F32)
            nc.scalar.activation(actwarm[:], ones4[:], EXP, bias=0.0, scale=1.0)

            qTn, kTn, vpt = [], [], []
            for n in range(NQT):
                tq = acts.tile([128, 2, QT], MDT, tag=f"qt{n}")
                tk = acts.tile([128, 2, QT], MDT, tag=f"kt{n}")
                qTn.append(tq)
                kTn.append(tk)
            for t in range(NKT):
                tv = acts.tile([128, HPC, DH + 1], MDT, tag=f"vp{t}")
                vpt.append(tv)

            def v_evac(t, src_ps, on_scalar):
                # fold the key-validity indicator into V and the
                # denominator column. The indicator stays off GPSIMD
                # (it flushes small values to zero on hardware).
                src = src_ps.rearrange("p (h d) -> p h d", h=HPC)
                if on_scalar:
                    nc.scalar.mul(vpt[t][:, :, 0:DH], src, dm01[:, t:t + 1])
                else:
                    nc.vector.tensor_scalar_mul(
                        vpt[t][:, :, 0:DH], src, dm01[:, t:t + 1])
                nc.vector.tensor_scalar_mul(
                    vpt[t][:, :, DH], ones4[:], dm01e[:, t:t + 1])

            # ---- Q^T and K^T project TOGETHER, chunk-major over the
            # contraction, in two passes sized to PSUM: pass A holds
            # the first two q tiles of each (4 accumulators = 8 banks)
            # and keeps the PE streaming at the chunk-DMA rate with no
            # head-of-line blocking; pass B covers the trimmed last
            # tiles (all chunks are resident by then) ----
            def qk_pass(ns):
                accs = []
                pools = [psS, psS, psO, psO]
                pi = 0
                for n in ns:
                    for w, dsts in ((wq, qTn), (wk, kTn)):
                        pool = pools[pi]
                        accs.append((pool.tile(
                            [128, 2, QT], F32,
                            tag="s" if pool is psS else "o",
                            name=f"psP{pi}"), n, w, dsts))
                        pi += 1
                for kt in range(NCH):
                    last = kt == NCH - 1
                    for acc, n, w, dsts in accs:
                        nco = jcols(n)
                        q0 = QT * n
                        for m in range(2):
                            nc.tensor.matmul(
                                acc[:, m, 0:nco],
                                w[:, kt, 128 * m:128 * m + 128],
                                xTs[kt][:, q0:q0 + nco],
                                start=(kt == 0), stop=last,
                            )
                        if last:
                            # evacuate right after each stop (split
                            # DVE/ScalarE; GPSIMD cannot read PSUM)
                            if n % 2 == 0:
                                nc.vector.tensor_copy(
                                    dsts[n][:, :, 0:nco], acc[:, :, 0:nco])
                            else:
                                nc.scalar.copy(
                                    dsts[n][:, :, 0:nco], acc[:, :, 0:nco])

            qk_pass(list(range(min(2, NQT))))
            if NQT > 2:
                qk_pass(list(range(2, NQT)))

            # ---- attention + output projection per q tile ----
            def emit_scores(j, m, i):
                r = i - 4 * j
                # diagonal k tiles: columns [0, 128r) are fully
                # masked - never computed, exp'd, or consumed
                c0 = 128 * r if r > 0 else 0
                nco = jcols(j)
                ps_s = psS.tile([128, 2, QT], F32, tag="s", name="ps_s")
                for u in range(2):
                    p0 = 64 * u
                    nc.tensor.matmul(
                        ps_s[:, u, c0:nco],
                        kTn[i // 4][p0:p0 + 64, m,
                                    128 * (i % 4):128 * (i % 4) + 128],
                        qTn[j][p0:p0 + 64, m, c0:nco],
                        start=True, stop=True,
                    )
                wt = wtp.tile([128, 2, QT], MDT, tag="wt", name="wt")
                nc.scalar.activation(
                    wt[:, :, c0:nco], ps_s[:, :, c0:nco], EXP,
                    bias=0.0, scale=SCALE)
                if r >= 0:  # causal triangle at columns [128r, 128r+128)
                    # on DVE (hardware GpSimd reloads its op library
                    # when switching op types - keep it broadcast-only)
                    nc.vector.tensor_mul(
                        wt[:, :, c0:c0 + KT], wt[:, :, c0:c0 + KT],
                        tri[:, None, :].broadcast_to([KT, 2, KT]),
                    )
                return wt, c0

            def emit_av(j, m, i, o_ps, wt, c0):
                nco = jcols(j)
                for u in range(2):
                    nc.tensor.matmul(
                        o_ps[0:DH + 1, u, c0:nco],
                        vpt[i][:, 2 * m + u, :],
                        wt[:, u, c0:nco],
                        start=(i == 0), stop=(i == jkt(j) - 1),
                    )

            def norm_chain(j, m, o_ps, o_all):
                # r = 1 / (sums + 1e-30). The +1e-30 keeps the
                # reciprocal finite for padded-tail columns when the
                # whole packed sequence is empty.
                nco = jcols(j)
                rbs = []
                for u in range(2):
                    r0 = sm.tile([1, QT], F32, tag=f"r0{u}", name="r0")
                    nc.vector.tensor_scalar_add(
                        r0[:, 0:nco], o_ps[DH:DH + 1, u, 0:nco], 1e-30)
                    rf = sm.tile([1, QT], F32, tag=f"rf{u}", name="rf")
                    nc.vector.reciprocal_approx_fast(
                        out=rf[:, 0:nco], in_=r0[:, 0:nco])
                    rbs.append(rf)
                for u in range(2):
                    rb = sm.tile([64, QT], F32, tag=f"rb{u}", name="rb")
                    nc.gpsimd.partition_broadcast(
                        rb[:, 0:nco], rbs[u][:, 0:nco], channels=64)
                    rbs[u] = rb
                for u in range(2):
                    nc.vector.tensor_mul(
                        o_all[64 * u:64 * u + 64, m, 0:nco],
                        o_ps[0:DH, u, 0:nco], rbs[u][:, 0:nco],
                    )

            def out_proj_s(j, o_all, s, last, deferred=False):
                # one dt-pair per [128, 2, 512] pp accumulator: one
                # evacuation + one DMA per pair. Deferred pairs ride
                # the transient psS rotation (scores/V-chain slots);
                # boundary and tail pairs nest into the o_ps pool.
                nco = jcols(j)
                if deferred:
                    pp = psS.tile([128, 2, QT], F32, tag="s", name="pp")
                else:
                    pp = psO.tile([128, 2, QT], F32, tag="o", name="pp")
                for sub in range(2):
                    dt = 2 * s + sub
                    for kt in range(2):
                        nc.tensor.matmul(
                            pp[:, sub, 0:nco],
                            wp[:, kt, 128 * dt:128 * dt + 128],
                            o_all[:, kt, 0:nco],
                            start=(kt == 0), stop=(kt == 1),
                        )
                ob = obp.tile([128, 2, QT], MDT, tag="ob")
                # during attention all evacuations ride DVE so ScalarE
                # stays exp-only; on the last tile ScalarE has no exps
                # left and takes half of them
                if last and s % 2 == 1:
                    nc.scalar.copy(ob[:, :, 0:nco], pp[:, :, 0:nco])
                else:
                    nc.vector.tensor_copy(ob[:, :, 0:nco], pp[:, :, 0:nco])
                # stores ride the SP queue during attention (the
                # Activation queue must stay exp-only); the last tile
                # splits across both hardware queues
                dma_eng = nc.scalar if (last and s % 2 == 1) else nc.sync
                dma_eng.dma_start(
                    out=out_d[256 * s:256 * s + 256,
                              QT * j:QT * j + nco].rearrange(
                                  "(c p) q -> p c q", p=128),
                    in_=ob[:, :, 0:nco],
                )

            # Global (j, m, i) stream, two k tiles ahead: scores/exp
            # run ahead of the AV matmuls so the PE always has ready
            # work while ScalarE runs the exp; four units of j+1 are
            # pumped around out_proj(j).
            def units(j):
                return [(m, i) for m in range(2) for i in range(jkt(j))]

            ahead = []           # [(m, i, wt, c0)] scores not yet AV'd
            pumped = {}          # j -> units emitted so far

            def pump(j):
                n = pumped.get(j, 0)
                ulist = units(j)
                if n < len(ulist):
                    m, i = ulist[n]
                    wt, c0 = emit_scores(j, m, i)
                    ahead.append((m, i, wt, c0))
                    pumped[j] = n + 1

            # ---- V projection, chunk-major per group of t tiles.
            # Tiles 0..3 (j=0's needs) run upfront; the rest are
            # emitted one tile at a time INTO the attention stream as
            # PE filler: the attention inner loop is ScalarE-bound, so
            # the V chains ride in the PE's exp-wait gaps, which also
            # keeps the HAM clock-gate from re-throttling.
            def v_chain(ts, split_evac):
                npsv = (len(ts) + 1) // 2
                psV = [psS.tile([128, 2, QT], F32, tag="s", name="psV0"),
                       psS.tile([128, 2, QT], F32, tag="s", name="psV1")][:npsv]
                for kt in range(NCH):
                    lastc = kt == NCH - 1
                    for tt, t in enumerate(ts):
                        nc.tensor.matmul(
                            psV[tt // 2][:, tt % 2, 0:DC],
                            xTs[kt][:, 128 * t:128 * t + 128],
                            wv[:, kt, :],
                            start=(kt == 0), stop=lastc,
                        )
                        if lastc:
                            v_evac(t, psV[tt // 2][:, tt % 2, 0:DC],
                                   on_scalar=(split_evac and tt % 2 == 1))

            nup = min(4, NKT)
            vq = list(range(nup, NKT))  # deferred V tiles

            # j=0 starts with four units in flight like the
            # steady-state boundary hand-off; the upfront V chain sits
            # between the pumps so its matmuls overlap the first exps
            pump(0)
            pump(0)
            v_chain(list(range(0, nup)), split_evac=False)
            pump(0)
            pump(0)

            # Filler scheduling: deferred V tile t must land before the
            # first AV that reads it (unit (m=0, i=t) of the first j
            # with jkt(j) > t); emit it ~6 units earlier. Deferred
            # out_proj dt-pairs of j fill j+1's exp-wait gaps.
            ucum = []
            tot = 0
            for j in range(NQT):
                ucum.append(tot)
                tot += len(units(j))

            due = {}
            for t in vq:
                jf = 0
                while jf < NQT - 1 and jkt(jf) <= t:
                    jf += 1
                first_read = ucum[jf] + t  # (m=0, i=t)
                at = max(2, first_read - 6)
                due.setdefault(at, []).append(
                    lambda t=t: v_chain([t], split_evac=False))

            gunit = 0
            for j in range(NQT):
                U = units(j)
                o_all = sm.tile([128, 2, QT], MDT, tag="oall")
                o_ps = {}
                for n in range(len(U)):
                    # pump first: the next unit's scores reach ScalarE
                    # before any filler block occupies the PE
                    pump(j)
                    for thunk in due.pop(gunit, []):
                        thunk()
                    m, i, wt, c0 = ahead.pop(0)
                    if i == 0:
                        o_ps[m] = psO.tile([128, 2, QT], F32, tag="o",
                                           name=f"o_ps{m}")
                    emit_av(j, m, i, o_ps[m], wt, c0)
                    if i == jkt(j) - 1:
                        norm_chain(j, m, o_ps[m], o_all)
                    gunit += 1
                jn = j + 1
                if jn == NQT:
                    for s in range(D // 256):
                        out_proj_s(j, o_all, s, last=True)
                else:
                    # every unit stream is exp-latency-gated (full
                    # tiles by exp size, trimmed tiles by per-unit
                    # ACTIVATE+semaphore latency): emit only the first
                    # dt-pair at the boundary and defer the rest into
                    # j+1's stream as PE filler
                    pump(jn)
                    pump(jn)
                    out_proj_s(j, o_all, 0, last=False)
                    pump(jn)
                    pump(jn)
                    for s in range(1, D // 256):
                        at = gunit + 2 + 4 * (s - 1)
                        due.setdefault(at, []).append(
                            lambda j=j, o_all=o_all, s=s:
                                out_proj_s(j, o_all, s, last=False,
                                           deferred=True))
            for at in sorted(due):  # any stragglers (tiny P edge cases)
                for thunk in due[at]:
                    thunk()

    nc.finalize()
    return nc


def _pack_indices(data_mask):
    """Per-batch indices of unmasked tokens, the static padded length
    P (multiple of 512) and the last-tile column count LC."""
    dm = np.asarray(data_mask)
    idxs = [np.nonzero(dm[b])[0] for b in range(B)]
    max_n = max((len(ix) for ix in idxs), default=0)
    max_n = max(max_n, 1)
    P = max(QT, -(-max_n // QT) * QT)
    LC = -(-(max_n - (P - QT)) // KT) * KT
    return idxs, P, LC


def _make_in_maps(x, data_mask, Wq, Wk, Wv, Wp, P, LC, idxs, mm_dtype=None):
    if (mm_dtype or MM_DTYPE) == "bfloat16":
        import ml_dtypes
        mdt = ml_dtypes.bfloat16
    else:
        mdt = np.float32
    KV = P - QT + LC
    NKT = KV // KT
    x = np.asarray(x, np.float32)
    # single [128, 128] causal triangle (q' >= p), shared by every
    # diagonal k tile
    p = np.arange(KT)[:, None]
    q = np.arange(KT)[None, :]
    tri = (q >= p).astype(np.float32)

    def chunked(w):  # [1024, C] -> [128, 8, C] (p-major chunks)
        cdim = w.shape[1]
        return np.ascontiguousarray(
            w.reshape(NCH, 128, cdim).transpose(1, 0, 2).astype(mdt))

    in_maps = []
    for c in range(NC):
        b, g = divmod(c, HPC)
        sl = slice(DC * g, DC * g + DC)
        ix = idxs[b]
        nb = len(ix)
        # packed + zero-padded tokens; padded q/k/v rows are exactly 0
        xp = np.zeros((P, D), np.float32)
        xp[:nb] = x[b][ix]
        dmb = np.zeros(KV, np.float32)
        dmb[:min(nb, KV)] = 1.0
        wp_c = np.asarray(Wp, np.float32)[sl, :]  # [256, 1024]
        in_maps.append({
            "xT": np.ascontiguousarray(xp.T.astype(mdt)),
            "wq": chunked(np.asarray(Wq, np.float32)[:, sl]),
            "wk": chunked(np.asarray(Wk, np.float32)[:, sl]),
            "wv": chunked(np.asarray(Wv, np.float32)[:, sl]),
            "wp": np.ascontiguousarray(
                wp_c.reshape(DC // 128, 128, D).transpose(1, 0, 2).astype(mdt)),
            "dm01": np.ascontiguousarray(dmb.reshape(NKT, KT).T),
            "dm01e": np.ascontiguousarray(
                np.maximum(dmb.reshape(NKT, KT).T, 1e-5)),
            "tri": tri.astype(mdt),
        })
    return in_maps


def _postprocess(results, data_mask, bp, idxs):
    out = np.zeros((B, T, D), np.float32)
    bp = np.asarray(bp, np.float32)
    for b in range(B):
        acc = results[HPC * b]["outT"].astype(np.float32).copy()
        for g in range(1, HPC):
            acc += results[HPC * b + g]["outT"]
        ix = idxs[b]
        # scatter packed rows back; masked-query rows stay zero
        out[b, ix, :] = acc.T[:len(ix)]
        if np.any(bp):
            out[b, ix, :] += bp
    return out


def _numpy_reference(x, data_mask, Wq, bq, Wk, bk, Wv, bv, Wp, bp):
    # general fallback (only used when q/k/v biases are nonzero, which
    # does not happen for this problem's setup_inputs)
    x = np.asarray(x, np.float64)
    dm = np.asarray(data_mask) != 0
    q = (x @ np.asarray(Wq, np.float64) + np.asarray(bq, np.float64))
    k = (x @ np.asarray(Wk, np.float64) + np.asarray(bk, np.float64))
    v = (x @ np.asarray(Wv, np.float64) + np.asarray(bv, np.float64))
    q = q.reshape(B, T, H, DH).transpose(0, 2, 1, 3) * SCALE
    k = k.reshape(B, T, H, DH).transpose(0, 2, 1, 3)
    v = v.reshape(B, T, H, DH).transpose(0, 2, 1, 3)
    causal = np.tril(np.ones((T, T), bool))
    out = np.empty((B, T, D), np.float64)
    for b in range(B):
        mask = causal & dm[b][:, None] & dm[b][None, :]
        for h in range(H):
            s = q[b, h] @ k[b, h].T
            s = np.where(mask, s, -np.inf)
            s -= np.max(s, axis=-1, keepdims=True)
            w = np.exp(s)
            denom = w.sum(-1, keepdims=True)
            w = np.where(denom > 0, w / np.where(denom == 0, 1, denom), 0.0)
            w = np.nan_to_num(w)
            out[b, :, h * DH:(h + 1) * DH] = w @ v[b, h]
    out = out @ np.asarray(Wp, np.float64) + np.asarray(bp, np.float64)
    out *= dm[..., None]
    return out.astype(np.float32)


def kernel(x, data_mask, Wq, bq, Wk, bk, Wv, bv, Wp, bp):
    if any(np.any(np.asarray(v)) for v in (bq, bk, bv)):
        return _numpy_reference(x, data_mask, Wq, bq, Wk, bk, Wv, bv, Wp, bp)

    from concourse.bass_utils import run_bass_kernel_spmd

    idxs, P, LC = _pack_indices(data_mask)
    key = (P, LC)
    if key not in _cached:
        _cached[key] = _build_program(P, LC)
    nc = _cached[key]
    in_maps = _make_in_maps(x, data_mask, Wq, Wk, Wv, Wp, P, LC, idxs)
    res = run_bass_kernel_spmd(nc, in_maps, core_ids=list(range(NC)))
    return _postprocess(res.results, data_mask, bp, idxs)
